# revision 7
# baseline (speedup 1.0000x reference)
"""Trainium2 Bass kernel for BugLocalizationGNN (3-layer GAT + classifier).

v2: bf16 data path.  Nodes partitioned across 8 cores (6250 dst nodes each);
edges sharded by destination.  Per GAT layer:
  1. node-sharded dense matmul h = z @ W in bf16 (PE 1 cyc/row), fused
     per-head attention score columns s = h.a_src, d = h.a_dst via
     host-precomputed [W | W@As | W@Ad] weight blocks
  2. AllGather of the bf16 gather-table rows [h | s | pad] (1280B for the
     4-head layers, 512B for the single-head layer) into each core's HBM
  3. per-128-edge-chunk: dma_gather of source rows; one-hot dst-selection
     matrices (DVE is_equal) feed PE matmuls that accumulate both the
     weighted message sum (rhs = gathered h * ev) and the softmax
     denominator (rhs = ev), with ev = exp(leakyrelu(s[src]+d[dst]))
     (global-shift-free softmax — mathematically identical to the
     segment-max-shifted softmax; logits are bounded)
  4. alpha-normalize + (host-folded) BN + ELU, transpose via PE into the
     feature-major blocked layout the next layer's matmul consumes.

The int16 gather-index limit (< 32768) is handled with two table windows
A=[0, 32768) and B=[N-32768, N); edges with src in the overlap are assigned
per (core, tile) to balance chunk counts.  Chunk counts are uniform across
cores per (tile, window) so a single SPMD program fits every core.
"""

import numpy as np
import ml_dtypes

BF16 = ml_dtypes.bfloat16

P = 128
NCORES = 8
WIN = 32768
PAD_DST = 200.0   # dstcol value for padding lanes (never matches iota 0..127)
PAD_REP = 255     # dstrep value for padding lanes
ECLAMP = 30.0     # safety clamp on attention logits before exp


# ----------------------------------------------------------------------------
# host-side planning
# ----------------------------------------------------------------------------

class Plan:
    pass


def _plan_edges(N, edge_index):
    """Partition edges by dst across cores; build per-core uniform chunk
    structure and the gather-index / selection-matrix input arrays."""
    NL = N // NCORES
    T = (NL + P - 1) // P
    src = np.concatenate([edge_index[0].astype(np.int64), np.arange(N, dtype=np.int64)])
    dst = np.concatenate([edge_index[1].astype(np.int64), np.arange(N, dtype=np.int64)])
    winb = N - WIN  # base of window B; overlap region is [winb, WIN)

    core_of = dst // NL
    dloc = dst - core_of * NL
    tile_of = dloc // P
    lane_of = dloc - tile_of * P

    # per (core, tile): src-sorted edge lists + window-class counts
    tiles = [[None] * T for _ in range(NCORES)]
    a0 = np.zeros((NCORES, T), np.int64)
    b0 = np.zeros((NCORES, T), np.int64)
    ov = np.zeros((NCORES, T), np.int64)
    for k in range(NCORES):
        mk = core_of == k
        sk, tk, lk = src[mk], tile_of[mk], lane_of[mk]
        for t in range(T):
            mt = tk == t
            s_t, l_t = sk[mt], lk[mt]
            order = np.argsort(s_t, kind="stable")
            s_t, l_t = s_t[order], l_t[order]
            tiles[k][t] = (s_t, l_t)
            a0[k, t] = int((s_t < winb).sum())
            b0[k, t] = int((s_t >= WIN).sum())
            ov[k, t] = len(s_t) - a0[k, t] - b0[k, t]

    cdiv = lambda a, b: -(-a // b)
    tot = a0 + b0 + ov
    CH_A = np.zeros(T, np.int64)
    CH_B = np.zeros(T, np.int64)
    for t in range(T):
        ca_min = max(cdiv(int(a0[k, t]), P) for k in range(NCORES))
        cb_min = max(cdiv(int(b0[k, t]), P) for k in range(NCORES))
        ct = max(max(cdiv(int(tot[k, t]), P) for k in range(NCORES)),
                 ca_min + cb_min)
        ct = max(ct, 1)
        CH_A[t] = max(ca_min, ct - cb_min)  # give A the slack (A side larger)
        CH_B[t] = ct - CH_A[t]

    # group tiles in pairs; chunk sequence per group: A-run then B-run.
    groups = [tuple(range(g, min(g + 2, T))) for g in range(0, T, 2)]
    NCHUNK = int((CH_A + CH_B).sum())
    E_pad = NCHUNK * P

    # compile-time metadata shared by all cores
    chunk_meta = []   # per chunk: (tile, first, last)
    blocks = []       # flat list per dma_gather: (win, chunk0, nchunks)
    grp_meta = []     # per group: dict(c0, nch, runs=[(win, c0, nch, blocks)])
    counts = {t: 0 for t in range(T)}
    total_t = {t: int(CH_A[t] + CH_B[t]) for t in range(T)}
    chunk_tile_off = {}   # chunk idx -> (tile, offset within its (tile,win) run)
    gc = 0
    for grp in groups:
        gm = dict(grp=grp, c0=gc, runs=[])
        for win, chw in (("A", CH_A), ("B", CH_B)):
            nch = int(sum(chw[t] for t in grp))
            if nch == 0:
                continue
            rblocks = []
            for bb0 in range(0, nch, 8):
                blk = (win, gc + bb0, min(8, nch - bb0))
                rblocks.append(blk)
                blocks.append(blk)
            gm["runs"].append((win, gc, nch, rblocks))
            for t in grp:
                for c in range(int(chw[t])):
                    cm = counts[t]
                    chunk_meta.append((t, cm == 0, cm == total_t[t] - 1))
                    chunk_tile_off[gc] = (t, win, c)
                    counts[t] += 1
                    gc += 1
        gm["nch"] = gc - gm["c0"]
        grp_meta.append(gm)
    assert gc == NCHUNK

    # per-core arrays
    idx_cols = E_pad // 16
    idx16 = np.zeros((NCORES, P, idx_cols), np.int16)
    dstcol = np.full((NCORES, P, NCHUNK), PAD_DST, np.float32)
    dstrep = np.full((NCORES, P, E_pad), PAD_REP, np.uint8)

    for k in range(NCORES):
        flat_idx = np.zeros(E_pad, np.int16)
        flat_lane = np.full(E_pad, -1, np.int64)
        # choose per-tile split of overlap edges: fill A up to capacity
        for t in range(T):
            s_t, l_t = tiles[k][t]
            capA = int(CH_A[t]) * P
            capB = int(CH_B[t]) * P
            nA = min(int(a0[k, t] + ov[k, t]), capA)
            nA = max(nA, len(s_t) - capB)
            assert a0[k, t] <= nA <= a0[k, t] + ov[k, t]
            tiles[k][t] = (s_t, l_t, nA)
        gc = 0
        for grp in groups:
            for win, chw in (("A", CH_A), ("B", CH_B)):
                nch = int(sum(chw[t] for t in grp))
                if nch == 0:
                    continue
                for t in grp:
                    s_t, l_t, nA = tiles[k][t]
                    if win == "A":
                        s_w, l_w = s_t[:nA], l_t[:nA]
                    else:
                        s_w, l_w = s_t[nA:] - winb, l_t[nA:]
                    n = len(s_w)
                    o = gc * P
                    assert n <= int(chw[t]) * P
                    flat_idx[o:o + n] = s_w.astype(np.int16)
                    flat_lane[o:o + n] = l_w
                    gc += int(chw[t])
        # wrapped+replicated index layout per gather block
        for win, c0, nch in blocks:
            seg = flat_idx[c0 * P:(c0 + nch) * P]
            wrapped = seg.reshape(-1, 16).T            # [16, n/16]
            col0 = c0 * P // 16
            idx16[k, :, col0:col0 + wrapped.shape[1]] = np.tile(wrapped, (8, 1))
        lane = flat_lane.reshape(NCHUNK, P).T          # [P, NCHUNK]
        valid = lane >= 0
        dstcol[k][valid] = lane[valid].astype(np.float32)
        rep = np.where(flat_lane >= 0, flat_lane, PAD_REP).astype(np.uint8)
        dstrep[k] = np.tile(rep[None, :], (P, 1))

    pl = Plan()
    pl.N, pl.NL, pl.T = N, NL, T
    pl.CH_A, pl.CH_B = CH_A, CH_B
    pl.NCHUNK, pl.E_pad = NCHUNK, E_pad
    pl.groups, pl.chunk_meta, pl.blocks = groups, chunk_meta, blocks
    pl.grp_meta = grp_meta
    pl.winb = winb
    pl.idx16, pl.dstcol, pl.dstrep = idx16, dstcol, dstrep
    return pl


def _fold_bn(g, be, rm, rv, b, eps=1e-5):
    k = (g / np.sqrt(rv + eps)).astype(np.float64)
    c = (b.astype(np.float64) - rm) * k + be
    return k.astype(np.float32), c.astype(np.float32)


def _prep_weights(W, a_s, a_d, bias, g, be, rm, rv):
    """Host precompute: blocked bf16 [W | Ws | Wd] and folded BN constants.

    Returns Wblk [P, KT, NF+2H] bf16 with Wblk[p, a, :] = row a*P+p of
    [W | W@As | W@Ad], plus BN mult/add rows replicated to [P, FW] f32.
    """
    IN = W.shape[0]
    Hh, C = a_s.shape
    NF = Hh * C
    KT = IN // P
    del KT  # re-derived below
    KT = IN // P
    Wsd = np.zeros((IN, 2 * Hh), np.float64)
    for h in range(Hh):
        blk = W[:, h * C:(h + 1) * C].astype(np.float64)
        Wsd[:, h] = blk @ a_s[h].astype(np.float64)
        Wsd[:, Hh + h] = blk @ a_d[h].astype(np.float64)
    k, c = _fold_bn(np.asarray(g, np.float64), np.asarray(be, np.float64),
                    np.asarray(rm, np.float64), np.asarray(rv, np.float64),
                    np.asarray(bias, np.float64))
    kcols = k if len(k) == NF else np.tile(k, Hh)   # mean-head layer: k per C
    Wmain = W.astype(np.float64) * kcols[None, :]
    Wall = np.concatenate([Wmain, Wsd], axis=1)  # [IN, NF+2H]
    Wblk = np.ascontiguousarray(
        Wall.reshape(KT, P, NF + 2 * Hh).transpose(1, 0, 2)).astype(BF16)
    return Wblk, np.tile(c, (P, 1))


def _block_x(x, T):
    """[N_loc, IN] f32 -> [P, T, KT, P] bf16 with xb[p, t, a, j] = x[t*P+j, a*P+p]."""
    NLoc, IN = x.shape
    KT = IN // P
    xb = np.zeros((P, T, KT, P), BF16)
    xt = np.ascontiguousarray(x.T).astype(BF16)          # [IN, NLoc]
    full = xt.reshape(KT, P, NLoc)                        # [a, p, n]
    for t in range(T):
        n0, n1 = t * P, min((t + 1) * P, NLoc)
        xb[:, t, :, :n1 - n0] = full[:, :, n0:n1].transpose(1, 0, 2)
    return xb


# ----------------------------------------------------------------------------
# device program
# ----------------------------------------------------------------------------

def _build_program(pl, dims):
    import concourse.tile as tile
    from concourse import bacc, mybir

    f32 = mybir.dt.float32
    bf16 = mybir.dt.bfloat16
    i16 = mybir.dt.int16
    u8 = mybir.dt.uint8

    NL, T = pl.NL, pl.T
    layers = dims["layers"]   # list of dicts: IN, H, C, ROWW, concat
    HID = dims["HID"]

    nc = bacc.Bacc("TRN2", target_bir_lowering=False, debug=False,
                   num_devices=NCORES)

    def din(name, shape, dt=f32):
        return nc.dram_tensor(name, list(shape), dt, kind="ExternalInput").ap()

    x_blk = din("x_blk", (P, T * (layers[0]["IN"] // P) * P), bf16)
    eidx = din("eidx", pl.idx16.shape[1:], i16)
    dstcol = din("dstcol", pl.dstcol.shape[1:])
    dstrep_d = din("dstrep", pl.dstrep.shape[1:], u8)
    iota_row_d = din("iota_row", (P, P))
    iota_col_d = din("iota_col", (P, 1))
    Wblk_d, crep_d = [], []
    for li, L in enumerate(layers):
        KT = L["IN"] // P
        Wblk_d.append(din(f"Wblk{li}", (P, KT * (L["H"] * L["C"] + 2 * L["H"])), bf16))
        FW = L["H"] * L["C"] if L["concat"] else L["C"]
        crep_d.append(din(f"crep{li}", (P, FW)))
    Wc_d = din("Wc", (HID, 2), bf16)
    bcrep_d = din("bcrep", (P, 2))

    out_d = nc.dram_tensor("out", [NL, 2], f32, kind="ExternalOutput").ap()
    dbg = dims.get("debug", False)
    dbg_d = {}
    if dbg:
        for li, L in enumerate(layers):
            dbg_d[f"dbg_haug{li}"] = nc.dram_tensor(
                f"dbg_haug{li}", [NL, L["ROWW"]], bf16, kind="ExternalOutput").ap()
            if li + 1 < len(layers):
                KT2 = layers[li + 1]["IN"] // P
                dbg_d[f"dbg_zblk{li}"] = nc.dram_tensor(
                    f"dbg_zblk{li}", [P, T * KT2 * P], bf16,
                    kind="ExternalOutput").ap()

    # internal DRAM
    haug_loc, haug_full, zblk = [], [], []
    for li, L in enumerate(layers):
        haug_loc.append(nc.dram_tensor(f"haug_loc{li}", [NL, L["ROWW"]], bf16).ap())
        haug_full.append(nc.dram_tensor(f"haug_full{li}", [pl.N, L["ROWW"]], bf16,
                                        addr_space="Shared").ap())
        if li + 1 < len(layers):
            KT2 = layers[li + 1]["IN"] // P
            zblk.append(nc.dram_tensor(f"zblk{li}", [P, T * KT2 * P], bf16).ap())

    with tile.TileContext(nc) as tc:
        _emit(tc, nc, pl, dims, locals(), mybir)
    nc.compile()
    return nc


def _emit(tc, nc, pl, dims, refs, mybir):
    from contextlib import ExitStack
    from concourse.masks import make_identity

    f32 = mybir.dt.float32
    bf16 = mybir.dt.bfloat16
    u8 = mybir.dt.uint8
    AF = mybir.ActivationFunctionType
    OP = mybir.AluOpType

    NL, T, N = pl.NL, pl.T, pl.N
    layers = dims["layers"]
    x_blk, eidx, dstcol, dstrep_d = refs["x_blk"], refs["eidx"], refs["dstcol"], refs["dstrep_d"]
    iota_row_d, iota_col_d = refs["iota_row_d"], refs["iota_col_d"]
    Wblk_d, crep_d = refs["Wblk_d"], refs["crep_d"]
    Wc_d, bcrep_d, out_d = refs["Wc_d"], refs["bcrep_d"], refs["out_d"]
    haug_loc, haug_full, zblk = refs["haug_loc"], refs["haug_full"], refs["zblk"]

    ctx = ExitStack()
    with ctx:
        const = ctx.enter_context(tc.tile_pool(name="const", bufs=1))
        wpool = ctx.enter_context(tc.tile_pool(name="wpool", bufs=1))
        mm_in = ctx.enter_context(tc.tile_pool(name="mm_in", bufs=3))
        aug_pool = ctx.enter_context(tc.tile_pool(name="aug", bufs=3))
        gpool = ctx.enter_context(tc.tile_pool(name="gpool", bufs=3))
        rep_pool = ctx.enter_context(tc.tile_pool(name="rep", bufs=2))
        sel_pool = ctx.enter_context(tc.tile_pool(name="sel", bufs=4))
        wg_pool = ctx.enter_context(tc.tile_pool(name="wg", bufs=3))
        ev_pool = ctx.enter_context(tc.tile_pool(name="ev", bufs=2))
        post_pool = ctx.enter_context(tc.tile_pool(name="post", bufs=3))
        keep = ctx.enter_context(tc.tile_pool(name="keep", bufs=1))

        # ---- resident constants
        iota_row = const.tile([P, P], f32)
        nc.sync.dma_start(out=iota_row[:], in_=iota_row_d[:])
        iota_col = const.tile([P, 1], f32)
        nc.sync.dma_start(out=iota_col[:], in_=iota_col_d[:])
        ident_f = const.tile([P, P], f32)
        make_identity(nc, ident_f[:])
        ident = const.tile([P, P], bf16)
        nc.vector.tensor_copy(ident[:], ident_f[:])
        idx_sb = const.tile(list(pl.idx16.shape[1:]), mybir.dt.int16)
        nc.sync.dma_start(out=idx_sb[:], in_=eidx[:])
        dstcol_sb = const.tile(list(pl.dstcol.shape[1:]), f32)
        nc.sync.dma_start(out=dstcol_sb[:], in_=dstcol[:])

        Wblk_sb, crep_sb = [], []
        for li, L in enumerate(layers):
            KT = L["IN"] // P
            WD = L["H"] * L["C"] + 2 * L["H"]
            wm = wpool.tile([P, KT, WD], bf16, tag=f"wm{li}")
            nc.sync.dma_start(out=wm[:], in_=Wblk_d[li][:])
            Wblk_sb.append(wm)
            FW = L["H"] * L["C"] if L["concat"] else L["C"]
            ct = wpool.tile([P, FW], f32, tag=f"c{li}")
            nc.sync.dma_start(out=ct[:], in_=crep_d[li][:])
            crep_sb.append(ct)
        Wc_sb = wpool.tile([P, 2], bf16)
        nc.sync.dma_start(out=Wc_sb[:], in_=Wc_d[:])
        bcrep_sb = wpool.tile([P, 2], f32)
        nc.sync.dma_start(out=bcrep_sb[:], in_=bcrep_d[:])

        d_loc = [keep.tile([P, T * L["H"]], bf16, tag=f"dloc{li}",
                           name=f"dloc{li}")
                 for li, L in enumerate(layers)]
        for dl in d_loc:
            nc.vector.memset(dl[:], 0.0)

        def rows_of(t):
            return min(P, NL - t * P)

        # ------------------------------------------------------------------
        def matmul_phase(li, mm_ps, mm_sd_ps):
            L = layers[li]
            H, C, IN, ROWW = L["H"], L["C"], L["IN"], L["ROWW"]
            NF = H * C
            KT = IN // P
            zin = x_blk if li == 0 else zblk[li - 1]
            zin3 = zin[:].rearrange("p (t a j) -> p t a j", t=T, a=KT)
            for t in range(T):
                mt = rows_of(t)
                lhs = mm_in.tile([P, KT, P], bf16, tag="lhs")
                nc.sync.dma_start(out=lhs[:], in_=zin3[:, t])
                ps1 = mm_ps.tile([P, NF], f32)
                ps2 = mm_sd_ps.tile([P, 2 * H], f32)
                for kk in range(KT):
                    nc.tensor.matmul(out=ps1[:mt, :], lhsT=lhs[:, kk, :mt],
                                     rhs=Wblk_sb[li][:, kk, :NF],
                                     start=(kk == 0), stop=(kk == KT - 1))
                    nc.tensor.matmul(out=ps2[:mt, :], lhsT=lhs[:, kk, :mt],
                                     rhs=Wblk_sb[li][:, kk, NF:],
                                     start=(kk == 0), stop=(kk == KT - 1))
                aug = aug_pool.tile([P, ROWW], bf16, tag="aug")
                nc.scalar.activation(out=aug[:mt, :NF], in_=ps1[:mt, :],
                                     func=AF.Copy)
                nc.vector.tensor_copy(out=aug[:mt, NF:NF + H], in_=ps2[:mt, :H])
                if ROWW > NF + H:
                    nc.vector.memset(aug[:mt, NF + H:], 0.0)
                nc.vector.tensor_copy(
                    out=d_loc[li][:mt, t * H:(t + 1) * H],
                    in_=ps2[:mt, H:2 * H])
                nc.sync.dma_start(out=haug_loc[li][t * P:t * P + mt, :],
                                  in_=aug[:mt, :])

            if dims.get("nocc"):
                nc.sync.dma_start(out=haug_full[li][:NL, :], in_=haug_loc[li][:])
            else:
                nc.gpsimd.collective_compute(
                    "AllGather", mybir.AluOpType.bypass,
                    replica_groups=[list(range(NCORES))],
                    ins=[haug_loc[li][:].opt()],
                    outs=[haug_full[li][:].opt()],
                )

        # ------------------------------------------------------------------
        def agg_phase(li, agg_ps, den_ps, dexp_ps, tr_ps):
            L = layers[li]
            H, C, ROWW = L["H"], L["C"], L["ROWW"]
            NF = H * C
            for gm in pl.grp_meta:
                grp = gm["grp"]
                ps_main = {t: agg_ps.tile([P, NF], f32, tag="agm", name=f"agm{t}")
                           for t in grp}
                ps_den = {t: den_ps.tile([P, H], f32, tag="den", name=f"den{t}")
                          for t in grp}
                g_c0, g_nch = gm["c0"], gm["nch"]
                rep_sb = rep_pool.tile([P, g_nch * P], u8, tag="rep")
                nc.sync.dma_start(out=rep_sb[:],
                                  in_=dstrep_d[:, g_c0 * P:(g_c0 + g_nch) * P])

                for win, c0, nch, rblocks in gm["runs"]:
                    # gathers for this run
                    gtiles = []
                    base = 0 if win == "A" else pl.winb
                    for bwin, bc0, bn in rblocks:
                        gt = gpool.tile([P, bn, ROWW], bf16, tag="G")
                        if "gather" in AB:
                            gtiles.append((bc0, bn, gt)); continue
                        nc.gpsimd.dma_gather(
                            out_ap=gt[:],
                            in_ap=haug_full[li][base:base + min(WIN, N), :],
                            idxs_ap=idx_sb[:, bc0 * P // 16:(bc0 + bn) * P // 16],
                            num_idxs=bn * P, num_idxs_reg=bn * P,
                            elem_size=ROWW)
                        gtiles.append((bc0, bn, gt))
                    # d per edge for the run (one-hot dst select matmuls)
                    psd = dexp_ps.tile([P, nch * H], f32, tag="dexp")
                    selT_all = sel_pool.tile([P, nch, P], bf16, tag="selTa",
                                             name="selTa")
                    sel_all = sel_pool.tile([P, nch, P], bf16, tag="sela",
                                            name="sela")
                    nc.vector.tensor_scalar(
                        out=selT_all[:],
                        in0=rep_sb[:, (c0 - g_c0) * P:(c0 - g_c0 + nch) * P],
                        scalar1=iota_col[:], scalar2=None, op0=OP.is_equal)
                    nc.vector.tensor_tensor(
                        out=sel_all[:],
                        in0=iota_row[:].rearrange("p (o m) -> p o m", o=1)
                            .to_broadcast([P, nch, P]),
                        in1=dstcol_sb[:, c0:c0 + nch]
                            .rearrange("p (n o) -> p n o", o=1)
                            .to_broadcast([P, nch, P]),
                        op=OP.is_equal)
                    for ci in range(nch):
                        if "dexp" in AB:
                            break
                        gc = c0 + ci
                        t = pl.chunk_meta[gc][0]
                        nc.tensor.matmul(out=psd[:, ci * H:(ci + 1) * H],
                                         lhsT=selT_all[:, ci],
                                         rhs=d_loc[li][:, t * H:(t + 1) * H],
                                         start=True, stop=True)
                    # batched e-values for the run: ev = exp(lrelu(s + d))
                    ev = ev_pool.tile([P, nch * H], f32, tag="ev")
                    sv = ev_pool.tile([P, nch * H], bf16, tag="sv")
                    for (bc0, bn, gt) in gtiles:
                        nc.vector.tensor_copy(
                            out=sv[:, (bc0 - c0) * H:(bc0 - c0 + bn) * H]
                                .rearrange("p (b h) -> p b h", h=H),
                            in_=gt[:, :, NF:NF + H])
                    nc.vector.tensor_tensor(out=ev[:], in0=psd[:], in1=sv[:],
                                            op=OP.add)
                    nc.vector.scalar_tensor_tensor(
                        out=ev[:], in0=ev[:], scalar=0.2, op0=OP.mult,
                        op1=OP.max, in1=ev[:])
                    evb = ev_pool.tile([P, nch * H], bf16, tag="evb")
                    nc.scalar.activation(out=evb[:], in_=ev[:], func=AF.Exp)
                    # weighted rows: wg = h * ev (batched per gather block)
                    for (bc0, bn, gt) in gtiles:
                        wg = wg_pool.tile([P, bn, H, C], bf16, tag="wg",
                                          name=f"wg{bc0}")
                        if "wg" in AB:
                            nc.vector.memset(wg[:, 0, 0, 0:1], 1.0)
                        else:
                            nc.vector.tensor_tensor(
                                out=wg[:],
                                in0=gt[:, :, :NF].rearrange(
                                    "p b (h c) -> p b h c", h=H),
                                in1=evb[:, (bc0 - c0) * H:(bc0 - c0 + bn) * H]
                                    .rearrange("p (b h c) -> p b h c", h=H, c=1)
                                    .to_broadcast([P, bn, H, C]),
                                op=OP.mult)
                        for j in range(bn):
                            gc = bc0 + j
                            ci = gc - c0
                            t, first, last = pl.chunk_meta[gc]
                            ti = grp.index(t)
                            if "aggmm" in AB:
                                continue
                            nc.tensor.matmul(
                                out=ps_main[t][:],
                                lhsT=sel_all[:, ci],
                                rhs=wg[:, j].rearrange("p h c -> p (h c)"),
                                start=first, stop=last)
                            nc.tensor.matmul(
                                out=ps_den[t][:],
                                lhsT=sel_all[:, ci],
                                rhs=evb[:, ci * H:(ci + 1) * H],
                                start=first, stop=last)
                # ---- post-processing for the group's tiles
                for t in grp:
                    ti = grp.index(t)
                    mt = rows_of(t)
                    FW = NF if L["concat"] else C
                    rc = post_pool.tile([P, H], f32, tag="rc")
                    nc.vector.reciprocal(rc[:], ps_den[t][:])
                    zt = post_pool.tile([P, FW], f32, tag="zt")
                    nc.vector.tensor_tensor(
                        out=zt[:].rearrange("p (h c) -> p h c", c=C),
                        in0=ps_main[t][:].rearrange("p (h c) -> p h c", c=C),
                        in1=rc[:, :FW // C].rearrange("p (h c) -> p h c", c=1)
                            .to_broadcast([P, FW // C, C]),
                        op=OP.mult)
                    nc.vector.tensor_tensor(out=zt[:], in0=zt[:], in1=crep_sb[li][:],
                                            op=OP.add)
                    mneg = post_pool.tile([P, FW], f32, tag="mneg")
                    nc.vector.tensor_scalar(out=mneg[:], in0=zt[:], scalar1=0.0,
                                            scalar2=None, op0=OP.min)
                    nc.scalar.activation(out=mneg[:], in_=mneg[:], func=AF.Exp)
                    zf = post_pool.tile([P, FW], bf16, tag="zf")
                    nc.vector.scalar_tensor_tensor(
                        out=zf[:], in0=mneg[:], scalar=-1.0,
                        op0=OP.add, op1=OP.max, in1=zt[:])
                    if li + 1 < len(layers):
                        # transpose to blocked feature-major for next matmul
                        KT2 = layers[li + 1]["IN"] // P
                        zb3 = zblk[li][:].rearrange("p (t a j) -> p t a j",
                                                    t=T, a=KT2)
                        zcall = post_pool.tile([P, KT2, P], bf16, tag="zcall")
                        for h in range(FW // P):
                            pt = tr_ps.tile([P, P], bf16, tag="tr")
                            nc.tensor.matmul(out=pt[:], lhsT=zf[:, h * P:(h + 1) * P],
                                             rhs=ident[:], is_transpose=True,
                                             start=True, stop=True)
                            nc.scalar.activation(out=zcall[:, h], in_=pt[:],
                                                 func=AF.Copy)
                        nc.sync.dma_start(out=zb3[:, t, :, :mt],
                                          in_=zcall[:, :, :mt])
                    else:
                        # classifier
                        pt = tr_ps.tile([P, P], bf16, tag="tr")
                        nc.tensor.matmul(out=pt[:], lhsT=zf[:, :P], rhs=ident[:],
                                         is_transpose=True, start=True, stop=True)
                        zc = post_pool.tile([P, P], bf16, tag="zcr")
                        nc.scalar.activation(out=zc[:], in_=pt[:], func=AF.Copy)
                        pc = den_ps.tile([P, 2], f32, tag="pc")
                        nc.tensor.matmul(out=pc[:mt, :], lhsT=zc[:, :mt], rhs=Wc_sb[:],
                                         start=True, stop=True)
                        ot = post_pool.tile([P, 2], f32, tag="ot")
                        nc.vector.tensor_tensor(out=ot[:mt, :], in0=pc[:mt, :],
                                                in1=bcrep_sb[:mt, :], op=OP.add)
                        nc.sync.dma_start(out=out_d[t * P:t * P + mt, :],
                                          in_=ot[:mt, :])

        AB = dims.get("ablate", set())
        dbg_d = refs.get("dbg_d", {})
        for _rep in range(dims.get("reps", 1)):
          for li in range(len(layers)):
            with tc.tile_pool(name=f"mm_ps{li}", bufs=2, space="PSUM") as mm_ps, \
                 tc.tile_pool(name=f"mm_sd_ps{li}", bufs=2, space="PSUM") as mm_sd_ps:
                matmul_phase(li, mm_ps, mm_sd_ps)
            if dbg_d:
                nc.sync.dma_start(out=dbg_d[f"dbg_haug{li}"][:],
                                  in_=haug_loc[li][:])
            last = li + 1 == len(layers)
            with tc.tile_pool(name=f"agg_ps{li}", bufs=2, space="PSUM") as agg_ps, \
                 tc.tile_pool(name=f"den_ps{li}", bufs=2, space="PSUM") as den_ps, \
                 tc.tile_pool(name=f"dexp_ps{li}", bufs=1 if last else 2, space="PSUM") as dexp_ps, \
                 tc.tile_pool(name=f"tr_ps{li}", bufs=1 if last else 2, space="PSUM") as tr_ps:
                agg_phase(li, agg_ps, den_ps, dexp_ps, tr_ps)
            if dbg_d and li + 1 < len(layers):
                nc.sync.dma_start(out=dbg_d[f"dbg_zblk{li}"][:], in_=zblk[li][:])


# ----------------------------------------------------------------------------
# entry point
# ----------------------------------------------------------------------------

def _layer_dims(IN, H, C, concat):
    used = H * C + H                 # h columns | s columns
    roww = -(-used * 2 // 256) * 128  # pad row to multiple of 256 bytes (bf16)
    return dict(IN=IN, H=H, C=C, concat=concat, ROWW=roww, AUGW=used)


def build_all(x, edge_index, W1, a1s, a1d, b1, g1, be1, rm1, rv1,
              W2, a2s, a2d, b2, g2, be2, rm2, rv2,
              W3, a3s, a3d, b3, g3, be3, rm3, rv3, Wc, bc, debug=False,
              nocc=False, ablate=(), reps=1):
    x = np.asarray(x)
    N, IN = x.shape
    HID = W3.shape[1]
    H = a1s.shape[0]
    pl = _plan_edges(N, np.asarray(edge_index))
    layers = [
        _layer_dims(IN, H, W1.shape[1] // H, True),
        _layer_dims(W1.shape[1], H, W2.shape[1] // H, True),
        _layer_dims(W2.shape[1], 1, W3.shape[1], False),
    ]
    dims = dict(layers=layers, HID=HID, debug=debug, nocc=nocc,
                ablate=set(ablate), reps=reps)

    Wb1, c1 = _prep_weights(W1, a1s, a1d, b1, g1, be1, rm1, rv1)
    Wb2, c2 = _prep_weights(W2, a2s, a2d, b2, g2, be2, rm2, rv2)
    Wb3, c3 = _prep_weights(W3, a3s, a3d, b3, g3, be3, rm3, rv3)

    iota_row = np.tile(np.arange(P, dtype=np.float32), (P, 1))
    iota_col = np.arange(P, dtype=np.float32).reshape(P, 1)

    in_maps = []
    for k in range(NCORES):
        xb = _block_x(x[k * pl.NL:(k + 1) * pl.NL], pl.T)
        m = dict(
            x_blk=np.ascontiguousarray(xb.reshape(P, -1)),
            eidx=pl.idx16[k], dstcol=pl.dstcol[k], dstrep=pl.dstrep[k],
            iota_row=iota_row, iota_col=iota_col,
            Wblk0=np.ascontiguousarray(Wb1.reshape(P, -1)), crep0=c1,
            Wblk1=np.ascontiguousarray(Wb2.reshape(P, -1)), crep1=c2,
            Wblk2=np.ascontiguousarray(Wb3.reshape(P, -1)), crep2=c3,
            Wc=np.asarray(Wc).astype(BF16),
            bcrep=np.tile(np.asarray(bc, np.float32), (P, 1)),
        )
        in_maps.append(m)

    nc = _build_program(pl, dims)
    return nc, in_maps, pl


def kernel(**inputs):
    from concourse.bass_utils import run_bass_kernel_spmd
    nc, in_maps, pl = build_all(**inputs)
    res = run_bass_kernel_spmd(nc, in_maps, core_ids=list(range(NCORES)))
    out = np.concatenate([res.results[k]["out"] for k in range(NCORES)], axis=0)
    return out.astype(np.float32)
